# revision 1
# baseline (speedup 1.0000x reference)
"""Trainium2 Bass kernel for the pairwise-alignment CRF loss (nn_CRFLoss).

Strategy
--------
Data parallel: batch 64 -> 8 cores x 8 batches. Per core, the log-domain
wavefront DP is reformulated as a probability-domain row sweep:

    M[i,j] = Em[i,j] * (U[i-1,j-1] + wh)
    X[i,j] = Ex[i,j] *  V[i-1,j]
    Y[i,j] = Ey[i,j] * (W02*M[i,j-1] + W12*X[i,j-1] + W22*Y[i,j-1])
    U = W00*M + W10*X + W20*Y ;  V = W01*M + W11*X + W21*Y

The in-row Y recurrence is a first-order linear scan handled by the DVE
TensorTensorScan instruction. Rows are swept with a skewed wavefront over
15 column chunks of 26 (partition p = b*16 + k holds batch b, chunk k;
slot k=0 is a constant-zero feeder), so chunk k processes row t-(k-1) at
step t and all cross-chunk halos travel one partition per step via a
single stream_shuffle. Each partition carries its own scale sigma_p
(rescaled every 8 steps, exact cross-chunk ratio correction rho), and the
Z mass is folded into per-window log-space accumulators, combined by a
final log-sum-exp. The gold-path score is gathered on device with
indirect DMA using host-computed element offsets.
"""

import sys

sys.path.insert(0, "/opt/trn_rl_repo")

import numpy as np

# ---------------- fixed problem geometry ----------------
B_FULL, XDIM, YDIM, LPATH = 64, 384, 384, 512
NCORES = 8
BPC = 8                   # batches per core
NK = 16                   # partition slots per batch (slot 0 = zero feeder)
NCH = 15                  # column chunks
CW = 26                   # chunk width (15*26 = 390 >= 384)
NP = BPC * NK             # 128 partitions
TSTEPS = 400              # wavefront steps (384 rows + 15 skew, rounded to 25 E-blocks)
BLK = 16                  # steps per emission block
NBLK = TSTEPS // BLK
REVERY = 10               # rescale / Z-fold cadence
NW = TSTEPS // REVERY     # Z windows
ROWS_P, COLS_P = 416, 418  # padded obs (top 14, left 27, value NEG)
NEGV = -1.0e4
SB = ROWS_P * COLS_P * 3   # padded obs strides (elements)
SR = COLS_P * 3

# state tile columns: [U 0:27 | B 27:54 | Y'' 54:81 | V 81:107 | h 107 | zacc 108:134]
US, BS, YS, VS, HC, ZS, STW = 0, 27, 54, 81, 107, 108, 134
RESC = 108  # rescaled column range [0:108)
ROWW = TSTEPS * 81 + 64  # skew tensor row width (E data + T6 table tail)

# consts tile columns
C_CA, C_CBETA, C_CGAMMA, C_CDELTA, C_CBQ, C_BM, C_BX, C_BY, C_TMT, C_H0, C_T9 = range(11)
NCONST = 12

_PROGRAM = None


def _build_program():
    import concourse.bass as bass
    import concourse.bacc as bacc
    import concourse.mybir as mybir
    import concourse.tile as tile

    f32 = mybir.dt.float32
    i32 = mybir.dt.int32
    Op = mybir.AluOpType
    AF = mybir.ActivationFunctionType

    nc = bacc.Bacc(
        "TRN2",
        target_bir_lowering=False,
        debug=False,
        enable_asserts=False,
        num_devices=NCORES,
    )

    skew = nc.dram_tensor("skew", [NP, ROWW], f32, kind="ExternalInput")
    consts = nc.dram_tensor("consts", [NP, NCONST], f32, kind="ExternalInput")
    gated = nc.dram_tensor("gate", [NP, TSTEPS], f32, kind="ExternalInput")
    maskyd = nc.dram_tensor("masky", [NP, CW], f32, kind="ExternalInput")
    offsd = nc.dram_tensor("offs", [NP, 32], i32, kind="ExternalInput")
    ctrd = nc.dram_tensor("ctr", [16, BPC], f32, kind="ExternalInput")
    identd = nc.dram_tensor("ident", [NP, NP], f32, kind="ExternalInput")
    idt8d = nc.dram_tensor("ind8", [NP, BPC], f32, kind="ExternalInput")
    lossd = nc.dram_tensor("loss", [BPC], f32, kind="ExternalOutput")

    # stream_shuffle mask: within each 16-slot group, slot k reads slot k-1
    # (slot 0 reads itself -> stays zero).
    shmask = [(i if i % NK == 0 else i - 1) for i in range(32)]

    def colap(ap, start, step, count):
        return bass.AP(tensor=ap.tensor, offset=ap.offset + start,
                       ap=[ap.ap[0], [step, count]])

    with tile.TileContext(nc) as tc:
        with (
            tc.tile_pool(name="persist", bufs=1) as pp,
            tc.tile_pool(name="eblk", bufs=2) as ep,
            tc.tile_pool(name="tmp", bufs=2) as wp,
            tc.tile_pool(name="qps", bufs=2, space="PSUM") as qp,
        ):
            st = pp.tile([NP, STW], f32)
            aw = pp.tile([NP, NW], f32)
            gt = pp.tile([NP, TSTEPS], f32)
            mk = pp.tile([NP, CW], f32)
            cs = pp.tile([NP, NCONST], f32)
            idt = pp.tile([NP, NP], f32)
            idt8 = pp.tile([NP, BPC], f32)
            lsg = pp.tile([NP, 1], f32)
            rho = pp.tile([NP, 1], f32)
            gth = pp.tile([NP, 32], f32)
            ofs = pp.tile([NP, 32], i32)
            ctr = pp.tile([16, BPC], f32)
            gsum = pp.tile([NP, 1], f32)
            ali8 = pp.tile([BPC, 1], f32)
            one1 = pp.tile([1, 1], f32)

            # ---- loads + init ----
            nc.sync.dma_start(out=cs, in_=consts.ap())
            nc.sync.dma_start(out=gt, in_=gated.ap())
            nc.sync.dma_start(out=mk, in_=maskyd.ap())
            nc.sync.dma_start(out=idt, in_=identd.ap())
            nc.sync.dma_start(out=idt8, in_=idt8d.ap())
            nc.sync.dma_start(out=ofs, in_=offsd.ap())
            nc.sync.dma_start(out=ctr, in_=ctrd.ap())
            nc.vector.memset(st[:, 0:STW], 0.0)
            nc.vector.memset(aw[:], -1.0e30)
            nc.vector.memset(lsg[:], 0.0)
            nc.vector.memset(rho[:], 1.0)
            nc.vector.memset(one1[:], 1.0)
            nc.vector.tensor_copy(st[:, HC:HC + 1], cs[:, C_H0:C_H0 + 1])

            # ---- alignment-score gathers (overlap with the DP) ----
            # HW indirect DMA: ONE offset per partition per instruction.
            obs_flat = bass.AP(tensor=skew.ap().tensor, offset=0,
                               ap=[[1, NP * ROWW], [1, 1]])
            for w in range(32):
                nc.gpsimd.indirect_dma_start(
                    out=gth[:, w:w + 1], out_offset=None, in_=obs_flat,
                    in_offset=bass.IndirectOffsetOnAxis(ap=ofs[:, w:w + 1], axis=0))
            nc.vector.reduce_sum(gsum[:], gth[:], axis=mybir.AxisListType.X)

            # ---- main skewed wavefront ----
            h_ap = st[:, HC:HC + 1]
            for blk in range(NBLK):
                t0 = blk * BLK
                eraw = ep.tile([NP, BLK * 81], f32, tag="eraw")
                et = ep.tile([NP, BLK * 81], f32, tag="et")
                nc.sync.dma_start(
                    out=eraw[:], in_=skew.ap()[:, t0 * 81:(t0 + BLK) * 81])
                err = eraw[:].rearrange("p (t s j) -> p t s j", t=BLK, s=3, j=27)
                etr = et[:].rearrange("p (t s j) -> p t s j", t=BLK, s=3, j=27)
                for s in range(3):
                    nc.scalar.activation(
                        etr[:, :, s, :], err[:, :, s, :], AF.Exp,
                        bias=cs[:, C_BM + s:C_BM + s + 1], scale=1.0)

                for tau in range(BLK):
                    t = t0 + tau
                    base = tau * 81
                    em = et[:, base + 1:base + 27]
                    ex = et[:, base + 27 + 1:base + 27 + 27]
                    eys = et[:, base + 54 + 0:base + 54 + 26]
                    eyu = et[:, base + 54 + 1:base + 54 + 27]

                    gp = wp.tile([NP, CW], f32, tag="gp")
                    hp = wp.tile([NP, CW], f32, tag="hp")
                    yh = wp.tile([NP, CW], f32, tag="yh")
                    w1 = wp.tile([NP, CW], f32, tag="w1")
                    zg = wp.tile([NP, CW], f32, tag="zg")

                    # G' = Em * (U_sh + h)
                    nc.vector.scalar_tensor_tensor(
                        gp[:], st[:, US:US + 26], h_ap, em, op0=Op.add, op1=Op.mult)
                    # halo shuffle + rho correction
                    hsrc = colap(st[:], US + 26, 27, 3)
                    hdst = colap(st[:], US, 27, 3)
                    nc.vector.stream_shuffle(hdst, hsrc, shmask)
                    nc.scalar.activation(hdst, hdst, AF.Copy, bias=0.0,
                                         scale=rho[:, 0:1])
                    # H' = Ex * V_prev (on GPSIMD to unload DVE)
                    nc.gpsimd.tensor_tensor(hp[:], ex, st[:, VS:VS + 26], op=Op.mult)
                    # B = cBq*G' + H'
                    nc.vector.scalar_tensor_tensor(
                        st[:, BS + 1:BS + 27], gp[:], cs[:, C_CBQ:C_CBQ + 1], hp[:],
                        op0=Op.mult, op1=Op.add)
                    # Y'' scan
                    nc.vector.tensor_tensor_scan(
                        st[:, YS + 1:YS + 27], eys, st[:, BS:BS + 26],
                        initial=st[:, YS:YS + 1], op0=Op.mult, op1=Op.add)
                    # Yhat = Ey' * Y''
                    nc.vector.tensor_tensor(yh[:], eyu, st[:, YS + 1:YS + 27], op=Op.mult)
                    # U update
                    nc.vector.scalar_tensor_tensor(
                        w1[:], yh[:], cs[:, C_CBETA:C_CBETA + 1], hp[:],
                        op0=Op.mult, op1=Op.add)
                    nc.vector.scalar_tensor_tensor(
                        st[:, US + 1:US + 27], w1[:], cs[:, C_CA:C_CA + 1], gp[:],
                        op0=Op.mult, op1=Op.add)
                    # V update: cDelta (= exp(T[2,1]+..), T[2,1] = -1e4) is 0,
                    # so V = cGamma*H' + G'
                    nc.vector.scalar_tensor_tensor(
                        st[:, VS:VS + 26], hp[:], cs[:, C_CGAMMA:C_CGAMMA + 1], gp[:],
                        op0=Op.mult, op1=Op.add)
                    # zacc += G' * gate_t (GPSIMD, gate broadcast along free)
                    gate_b = bass.AP(tensor=gt[:].tensor, offset=gt[:].offset + t,
                                     ap=[gt[:].ap[0], [0, CW]])
                    nc.gpsimd.tensor_tensor(zg[:], gp[:], gate_b, op=Op.mult)
                    nc.gpsimd.tensor_tensor(st[:, ZS:ZS + 26], st[:, ZS:ZS + 26],
                                            zg[:], op=Op.add)

                    if t % REVERY == REVERY - 1:
                        wix = t // REVERY
                        zm = wp.tile([NP, CW], f32, tag="zm")
                        zr = wp.tile([NP, 1], f32, tag="zr")
                        zq = wp.tile([NP, 1], f32, tag="zq")
                        fl = wp.tile([NP, 1], f32, tag="fl")
                        aa = wp.tile([NP, 1], f32, tag="aa")
                        rp = wp.tile([NP, 1], f32, tag="rp")
                        rc = wp.tile([NP, 1], f32, tag="rc")
                        rs = wp.tile([NP, 1], f32, tag="rs")
                        lr = wp.tile([NP, 1], f32, tag="lr")
                        # Z window fold
                        nc.gpsimd.tensor_tensor(zm[:], st[:, ZS:ZS + 26], mk[:], op=Op.mult)
                        nc.vector.reduce_sum(zr[:], zm[:], axis=mybir.AxisListType.X)
                        nc.vector.tensor_scalar(out=zq[:], in0=zr[:], scalar1=1.3e-38,
                                                scalar2=None, op0=Op.max)
                        nc.scalar.activation(aa[:], zq[:], AF.Ln)
                        nc.vector.tensor_tensor(aa[:], aa[:], lsg[:], op=Op.add)
                        nc.vector.tensor_scalar(out=fl[:], in0=zr[:], scalar1=1.4e-38,
                                                scalar2=None, op0=Op.is_lt)
                        nc.vector.scalar_tensor_tensor(
                            aw[:, wix:wix + 1], fl[:], -1.0e30, aa[:],
                            op0=Op.mult, op1=Op.add)
                        nc.gpsimd.memset(st[:, ZS:ZS + 26], 0.0)
                        # per-partition rescale
                        nc.vector.reduce_max(rp[:], st[:, 0:RESC], axis=mybir.AxisListType.X)
                        nc.vector.tensor_scalar(out=rp[:], in0=rp[:], scalar1=1e-30,
                                                scalar2=None, op0=Op.max)
                        nc.vector.reciprocal(rc[:], rp[:])
                        nc.vector.tensor_scalar(out=st[:, 0:RESC], in0=st[:, 0:RESC],
                                                scalar1=rc[:, 0:1], scalar2=None,
                                                op0=Op.mult)
                        nc.scalar.activation(lr[:], rp[:], AF.Ln)
                        nc.vector.tensor_tensor(lsg[:], lsg[:], lr[:], op=Op.add)
                        # rho *= r_{p-1} / r_p
                        nc.vector.stream_shuffle(rs[:], rp[:], shmask)
                        nc.vector.tensor_tensor(rs[:], rs[:], rc[:], op=Op.mult)
                        nc.vector.tensor_tensor(rho[:], rho[:], rs[:], op=Op.mult)

            # ---- endgame: Z per batch, then loss ----
            rmax = pp.tile([NP, 1], f32)
            dw = pp.tile([NP, NW], f32)
            sw = pp.tile([NP, 1], f32)
            apart = pp.tile([NP, 1], f32)
            nc.vector.reduce_max(rmax[:], aw[:], axis=mybir.AxisListType.X)
            nc.vector.tensor_scalar(out=dw[:], in0=aw[:], scalar1=rmax[:, 0:1],
                                    scalar2=None, op0=Op.subtract)
            nc.scalar.activation(dw[:], dw[:], AF.Exp)
            nc.vector.reduce_sum(sw[:], dw[:], axis=mybir.AxisListType.X)
            nc.scalar.activation(sw[:], sw[:], AF.Ln)
            nc.vector.tensor_tensor(apart[:], sw[:], rmax[:], op=Op.add)

            pt = qp.tile([1, NP], f32, space="PSUM")
            nc.tensor.matmul(pt[:], apart[:, 0:1], idt[:], start=True, stop=True)
            at = pp.tile([1, NP], f32)
            nc.vector.tensor_copy(at[:], pt[:])
            atv = at[:].rearrange("p (b k) -> p b k", b=BPC, k=NK)
            mb8 = pp.tile([1, BPC], f32)
            s8 = pp.tile([1, BPC], f32)
            nc.vector.reduce_max(mb8[:], atv, axis=mybir.AxisListType.X)
            mb8b = bass.AP(tensor=mb8[:].tensor, offset=mb8[:].offset,
                           ap=[mb8[:].ap[0], [1, BPC], [0, NK]])
            nc.vector.tensor_tensor(atv, atv, mb8b, op=Op.subtract)
            nc.scalar.activation(at[:], at[:], AF.Exp)
            nc.vector.reduce_sum(s8[:], atv, axis=mybir.AxisListType.X)
            nc.scalar.activation(s8[:], s8[:], AF.Ln)
            nc.vector.tensor_tensor(s8[:], s8[:], mb8[:], op=Op.add)

            p8 = qp.tile([BPC, 1], f32, space="PSUM")
            nc.tensor.matmul(p8[:], s8[:], one1[:], start=True, stop=True)
            # ali = per-batch obs gather sum + transition-count @ T6 values
            pa = qp.tile([BPC, 1], f32, space="PSUM")
            nc.tensor.matmul(pa[:], idt8[:], gsum[:], start=True, stop=False)
            nc.tensor.matmul(pa[:], ctr[:], cs[0:16, C_T9:C_T9 + 1],
                             start=False, stop=True)
            nc.vector.tensor_copy(ali8[:], pa[:])
            u8 = pp.tile([BPC, 1], f32)
            nc.vector.scalar_tensor_tensor(u8[:], ali8[:], -1.0, p8[:],
                                           op0=Op.mult, op1=Op.add)
            nc.vector.tensor_scalar(out=u8[:], in0=u8[:],
                                    scalar1=cs[0:BPC, C_TMT:C_TMT + 1],
                                    scalar2=None, op0=Op.add)
            loss_ap = bass.AP(tensor=lossd.ap().tensor, offset=0, ap=[[1, BPC], [1, 1]])
            nc.sync.dma_start(out=loss_ap, in_=u8[:])

    nc.compile()
    return nc


def _get_program():
    global _PROGRAM
    if _PROGRAM is None:
        _PROGRAM = _build_program()
    return _PROGRAM


def _prepare_inputs(observations, trans, P, alignments, maskX, maskY):
    """Host-side marshalling: pad/shard obs, bake masks/constants/offsets."""
    observations = np.asarray(observations, np.float32)
    trans = np.asarray(trans, np.float32)
    P = np.asarray(P, np.float32)
    alignments = np.asarray(alignments).astype(np.int64)
    maskX = np.asarray(maskX).astype(np.int64)
    maskY = np.asarray(maskY).astype(np.int64)

    T = (trans + P).astype(np.float64)
    cA = np.exp(T[1, 0] - T[0, 0])
    cBeta = np.exp(T[2, 0] + T[1, 2] - T[1, 0] - T[2, 2])
    cGamma = np.exp(T[1, 1] - T[0, 1])
    with np.errstate(over="ignore", under="ignore"):
        cDelta = np.exp(T[2, 1] + T[1, 2] - T[1, 1] - T[2, 2])
    cBq = np.exp(T[0, 2] - T[1, 2])
    h0 = np.exp(T[3, 0] - T[0, 0])
    consts = np.zeros((NP, NCONST), np.float32)
    consts[:, C_CA] = cA
    consts[:, C_CBETA] = cBeta
    consts[:, C_CGAMMA] = cGamma
    consts[:, C_CDELTA] = cDelta
    consts[:, C_CBQ] = cBq
    consts[:, C_BM] = T[0, 0]
    consts[:, C_BX] = T[0, 1]
    consts[:, C_BY] = T[2, 2]
    consts[:, C_TMT] = T[0, 4]
    consts[:, C_H0] = h0
    T9 = np.zeros(NP, np.float32)
    Tf = (trans + P).astype(np.float32)
    for s_ in range(3):
        for s2 in range(3):
            T9[s_ * 3 + s2] = Tf[s_, s2]
    consts[:, C_T9] = T9

    T6 = np.zeros((6, 6), np.float32)
    T6[:5, :5] = (trans + P).astype(np.float32)
    t6tab = np.zeros(64, np.float32)
    t6tab[:36] = T6.reshape(-1)

    ident = np.eye(NP, dtype=np.float32)
    ind8 = np.zeros((NP, BPC), np.float32)
    ind8[np.arange(NP), np.arange(NP) // NK] = 1.0

    kidx = np.arange(NP) % NK
    bidx = np.arange(NP) // NK
    # skew gather index grids (shared across cores)
    tg = np.arange(TSTEPS)
    rowg = tg[None, :] - kidx[:, None] + 15          # [NP, T] padded row index
    colg = (CW * kidx)[:, None] + np.arange(27)[None, :]  # [NP, 27] padded col

    in_maps = []
    for c in range(NCORES):
        bs = slice(c * BPC, (c + 1) * BPC)
        ob = observations[bs]
        mXc, mYc = maskX[bs], maskY[bs]
        obp = np.full((BPC, ROWS_P, COLS_P, 3), NEGV, np.float32)
        obp[:, 14:14 + XDIM, 27:27 + YDIM, :] = ob
        # skewed planar emission layout [p, t, s, j]
        sk = obp[bidx[:, None, None, None], rowg[:, :, None, None],
                 colg[:, None, None, :], np.arange(3)[None, None, :, None]]
        skewarr = np.zeros((NP, ROWW), np.float32)
        skewarr[:, :TSTEPS * 81] = sk.reshape(NP, TSTEPS * 81)
        skewarr[0, TSTEPS * 81:TSTEPS * 81 + 64] = t6tab

        # zacc gate: partition (b,k>=1) live at step t iff 0 <= t-(k-1) <= maskX-1
        tgrid = np.arange(TSTEPS)[None, :]
        r = tgrid - (kidx[:, None] - 1)
        gate = ((kidx[:, None] >= 1) & (r >= 0)
                & (r <= (mXc[bidx] - 1)[:, None])).astype(np.float32)
        # maskY: chunk col j global index <= maskY-1
        jj = (CW * (kidx[:, None] - 1) + np.arange(CW)[None, :])
        masky = ((kidx[:, None] >= 1)
                 & (jj <= (mYc[bidx] - 1)[:, None])).astype(np.float32)

        al = alignments[bs]
        x1, y1, s1 = al[..., 0], al[..., 1], al[..., 2]
        bloc = np.arange(BPC)[:, None]
        # path point (x,y,s): chunk k = (y-1)//26 + 1, local col = (y-1)%26 + 1,
        # step t = (x-1) + (k-1); element offset into skew[p, t, s, j]
        kk = (y1 - 1) // CW + 1
        jloc = (y1 - 1) % CW + 1
        tt = (x1 - 1) + (kk - 1)
        pp_ = bloc * NK + kk
        flatoff = pp_ * ROWW + (tt * 3 + s1) * 27 + jloc   # [BPC, 512]
        # arrange: value (b, l) -> partition b*16 + l%16, column l//16
        offs = np.zeros((NP, 32), np.int32)
        ll = np.arange(LPATH)
        for bb in range(BPC):
            offs[bb * NK + (ll % NK), ll // NK] = flatoff[bb]
        # transition pair counts
        pair = (s1[:, :-1] * 3 + s1[:, 1:]).astype(np.int64)   # [BPC, 511]
        ctr = np.zeros((16, BPC), np.float32)
        for bb in range(BPC):
            cnt = np.bincount(pair[bb], minlength=9)
            ctr[:9, bb] = cnt

        in_maps.append({
            "skew": skewarr,
            "consts": consts,
            "gate": np.ascontiguousarray(gate),
            "masky": np.ascontiguousarray(masky),
            "offs": offs,
            "ctr": ctr,
            "ident": ident,
            "ind8": ind8,
        })
    return in_maps


def kernel(observations, trans, P, alignments, maskX, maskY):
    from concourse import bass_utils

    in_maps = _prepare_inputs(observations, trans, P, alignments, maskX, maskY)
    nc = _get_program()
    res = bass_utils.run_bass_kernel_spmd(nc, in_maps, core_ids=list(range(NCORES)))
    out = np.concatenate([np.asarray(res.results[c]["loss"], np.float32)
                          for c in range(NCORES)])
    return out

